# revision 22
# baseline (speedup 1.0000x reference)
"""ALiBi causal self-attention on 8 TRN2 NeuronCores.

Sharding: core c handles batch b = c // 4 and the 4 heads {g, g+4, g+8, g+12}
with g = c % 4 (one head per "slope band" so per-core work balances and the
SPMD program is slot-uniform across cores).

Per-core program (single SPMD NEFF, fp32 storage, float32r matmuls):
  1. qkv^T = W^T x^T (2-head-paired matmuls, contraction over D on partitions)
  2. attention in transposed layout S^T[k, q] per 512-wide q-chunk with banded
     causal k-tiles; ALiBi bias via
       - slots 0/1 (largest slopes): exact bias+mask template added on DVE
       - slots 2/3: per-partition ACT bias exp(s + slope*(k - qmax)); the
         column factor exp(-slope*(q - qmax)) cancels in the softmax
         normalization, so it is never computed
     softmax denominator comes free from a ones-column appended to V
  3. normalize: broadcast the denominator row across partitions with a K=1
     matmul, reciprocal_approx_fast on the broadcast, one DVE multiply;
     then the out-projection (row-parallel); host sums the per-batch partials.
"""

import math

import numpy as np

B, L, D, H = 2, 2048, 1024, 16
DH = D // H  # 64
NCORES = 8
QC = 512  # q-chunk width
NQC = L // QC  # 4
NKT = L // 128  # 16 k-tiles
NLT = L // 128  # 16 l-tiles
NDC = D // 128  # 8 contraction chunks
NEG = -1.0e9
BAND_T = 34.0  # drop tiles whose best entry has bias < -BAND_T (exp < 2e-15)
TPATH = (0, 1)  # template-path slots
FPATH = (2, 3)  # free-path slots (ACT per-partition bias)

_NC = None


def _alibi_slopes(n):
    def pow2_slopes(m):
        start = 2 ** (-(2 ** -(math.log2(m) - 3)))
        return [start * start**i for i in range(m)]

    if math.log2(n).is_integer():
        return pow2_slopes(n)
    c = 2 ** math.floor(math.log2(n))
    return pow2_slopes(c) + _alibi_slopes(2 * c)[0::2][: n - c]


SLOPES = _alibi_slopes(H)
# Band window per slot = max over the heads that can occupy it (head 4j+g).
_W = [int(math.ceil(max(BAND_T / SLOPES[4 * j + g] for g in range(4)))) for j in range(4)]
WSLOT = [w if w <= L else 10**9 for w in _W]


def _kt_range(slot, qc):
    lo = max(0, -(-(QC * qc - 127 - WSLOT[slot]) // 128))
    return range(lo, 4 * qc + 4)


def _tmpl_geom(slot):
    """Template column geometry for a template-path slot: col j <-> c =
    j + c_min; tile (kt, qc) entry [pk, fq] reads c = fq - off with
    off = 128*kt - QC*qc."""
    offs = [128 * kt - QC * qc for qc in range(NQC) for kt in _kt_range(slot, qc)]
    c_min = 0 - max(offs)
    c_max = QC - 1 - min(offs)
    return c_min, c_max - c_min + 1


def _build_nc(with_qb, with_kb, with_vb):
    import concourse.bacc as bacc
    import concourse.mybir as mybir
    import concourse.tile as tile

    geom = {s: _tmpl_geom(s) for s in TPATH}

    f32 = mybir.dt.float32
    f32r = mybir.dt.float32r

    nc = bacc.Bacc("TRN2", target_bir_lowering=False, debug=False)

    xT = nc.dram_tensor("xT", [D, L], f32r, kind="ExternalInput")
    wq = nc.dram_tensor("wq", [D, 256], f32r, kind="ExternalInput")
    wk = nc.dram_tensor("wk", [D, 256], f32r, kind="ExternalInput")
    wv = nc.dram_tensor("wv", [D, 256], f32r, kind="ExternalInput")
    wo = nc.dram_tensor("wo", [256, D], f32r, kind="ExternalInput")
    tmpl = {
        s: nc.dram_tensor(f"tmpl{s}", [128, geom[s][1]], f32, kind="ExternalInput")
        for s in TPATH
    }
    abias = nc.dram_tensor(
        "abias", [128, len(FPATH), NQC, NKT], f32, kind="ExternalInput"
    )
    mdiag = nc.dram_tensor("mdiag", [128, 128], f32, kind="ExternalInput")
    vones = nc.dram_tensor("vones", [128, NKT, 4], f32r, kind="ExternalInput")
    pzero = nc.dram_tensor("pzero", [128, 384], f32r, kind="ExternalInput")
    if with_qb:
        qb = nc.dram_tensor("qb", [128, 2], f32, kind="ExternalInput")
    if with_kb:
        kb = nc.dram_tensor("kb", [128, 2], f32, kind="ExternalInput")
    if with_vb:
        vb = nc.dram_tensor("vb", [128, 256], f32, kind="ExternalInput")
    out = nc.dram_tensor("out", [L, D], f32, kind="ExternalOutput")

    Exp = mybir.ActivationFunctionType.Exp
    Identity = mybir.ActivationFunctionType.Identity
    add_op = mybir.AluOpType.add
    mult_op = mybir.AluOpType.mult

    with tile.TileContext(nc) as tc:
        with (
            tc.tile_pool(name="persist", bufs=1) as pp,
            tc.tile_pool(name="psum", bufs=1, space="PSUM") as ps,
        ):
            # ---- load constants / weights -------------------------------
            wq_sb = pp.tile([128, NDC, 256], f32r, tag="wq")
            wk_sb = pp.tile([128, NDC, 256], f32r, tag="wk")
            wv_sb = pp.tile([128, NDC, 256], f32r, tag="wv")
            wo_sb = pp.tile([128, 2, D], f32r, tag="wo")
            nc.sync.dma_start(wq_sb[:], wq.ap().rearrange("(c p) m -> p c m", p=128))
            nc.sync.dma_start(wk_sb[:], wk.ap().rearrange("(c p) m -> p c m", p=128))
            nc.sync.dma_start(wv_sb[:], wv.ap().rearrange("(c p) m -> p c m", p=128))
            nc.sync.dma_start(wo_sb[:], wo.ap().rearrange("(c p) m -> p c m", p=128))
            tmpl_sb = {}
            for s in TPATH:
                t = pp.tile([128, geom[s][1]], f32, tag=f"tmpl{s}", name=f"tmpl{s}")
                nc.sync.dma_start(t[:], tmpl[s][:])
                tmpl_sb[s] = t
            abias_sb = pp.tile([128, len(FPATH), NQC, NKT], f32, tag="abias")
            nc.sync.dma_start(abias_sb[:], abias[:])
            mdiag_sb = pp.tile([128, 128], f32, tag="mdiag")
            nc.sync.dma_start(mdiag_sb[:], mdiag[:])
            pzero_sb = pp.tile([128, 384], f32r, tag="pzero")
            nc.sync.dma_start(pzero_sb[:], pzero[:])
            vones_sb = pp.tile([128, NKT * 4], f32r, tag="vones")
            nc.sync.dma_start(vones_sb[:], vones.ap().rearrange("p a b -> p (a b)"))
            if with_qb:
                qb_sb = pp.tile([128, 2], f32, tag="qb")
                nc.sync.dma_start(qb_sb[:], qb[:])
            if with_kb:
                kb_sb = pp.tile([128, 2], f32, tag="kb")
                nc.sync.dma_start(kb_sb[:], kb[:])
            if with_vb:
                vb_sb = pp.tile([128, 256], f32, tag="vb")
                nc.sync.dma_start(vb_sb[:], vb[:])

            xp = tc.alloc_tile_pool(name="xt", bufs=1)
            xt_sb = []
            for dc in range(NDC):
                t = xp.tile([128, L], f32r, tag=f"xt{dc}", name=f"xt{dc}")
                nc.sync.dma_start(t[:], xT[dc * 128 : (dc + 1) * 128, :])
                xt_sb.append(t)

            # ---- persistent intermediates -------------------------------
            q2 = [pp.tile([128, L], f32r, tag=f"q2_{p}", name=f"q2_{p}") for p in range(2)]
            k2 = [pp.tile([128, L], f32r, tag=f"k2_{p}", name=f"k2_{p}") for p in range(2)]
            v_sb = pp.tile([128, NKT, 4, 65], f32r, tag="v")
            ot = [pp.tile([128, L], f32r, tag=f"ot_{p}", name=f"ot_{p}") for p in range(2)]

            # ---- QKV projections ---------------------------------------
            for pair in range(2):
                for qc in range(NQC):
                    sl = slice(qc * 512, qc * 512 + 512)
                    pq = ps.tile([128, 512], f32, tag="s", bufs=4, name="pq")
                    for hh in range(2):
                        hsl = slice(qc * 512 + hh * 256, qc * 512 + hh * 256 + 256)
                        for dc in range(NDC):
                            nc.tensor.matmul(
                                pq[:, hh * 256 : hh * 256 + 256],
                                wq_sb[:, dc, pair * 128 : pair * 128 + 128],
                                xt_sb[dc][:, hsl],
                                start=(dc == 0),
                                stop=(dc == NDC - 1),
                            )
                    if with_qb:
                        nc.scalar.activation(
                            q2[pair][:, sl], pq[:], Identity,
                            bias=qb_sb[:, pair : pair + 1],
                        )
                    else:
                        nc.scalar.copy(q2[pair][:, sl], pq[:])
                    pk_ = ps.tile([128, 512], f32, tag="s", bufs=4, name="pk_")
                    for hh in range(2):
                        hsl = slice(qc * 512 + hh * 256, qc * 512 + hh * 256 + 256)
                        for dc in range(NDC):
                            nc.tensor.matmul(
                                pk_[:, hh * 256 : hh * 256 + 256],
                                wk_sb[:, dc, pair * 128 : pair * 128 + 128],
                                xt_sb[dc][:, hsl],
                                start=(dc == 0),
                                stop=(dc == NDC - 1),
                            )
                    if with_kb:
                        nc.scalar.activation(
                            k2[pair][:, sl], pk_[:], Identity,
                            bias=kb_sb[:, pair : pair + 1],
                        )
                    else:
                        nc.scalar.copy(k2[pair][:, sl], pk_[:])

            for lt in range(NLT):
                pv_ = ps.tile([128, 256], f32, tag="s", bufs=4, name="pv_")
                for dc in range(NDC):
                    nc.tensor.matmul(
                        pv_[:],
                        xt_sb[dc][:, lt * 128 : lt * 128 + 128],
                        wv_sb[:, dc, :],
                        start=(dc == 0),
                        stop=(dc == NDC - 1),
                    )
                if with_vb:
                    nc.vector.tensor_tensor(pv_[:], pv_[:], vb_sb[:], add_op)
                nc.vector.tensor_copy(
                    v_sb[:, lt, 0:4, 0:64],
                    pv_.rearrange("p (s m) -> p s m", s=4),
                )
            nc.sync.dma_start(
                v_sb[:, :, :, 64:65], vones.ap().rearrange("p a (b o) -> p a b o", o=1)
            )
            xp.release()
            wp = tc.alloc_tile_pool(name="work", bufs=8)
            np_ = tc.alloc_tile_pool(name="norm", bufs=3)

            # ---- attention ----------------------------------------------
            for qc in range(NQC):
                qsl = slice(qc * QC, qc * QC + QC)
                for pair in range(2):
                    slots = (2 * pair, 2 * pair + 1)
                    kt_sets = {s: list(_kt_range(s, qc)) for s in slots}
                    pv_acc = {
                        (s, hh): ps.tile(
                            [128, 256], f32, tag="pv", bufs=4, name=f"pvacc{s}_{hh}"
                        )
                        for s in slots
                        for hh in range(2)
                    }
                    all_kts = sorted(set(kt_sets[slots[0]]) | set(kt_sets[slots[1]]))
                    WIN = 6
                    for wstart in range(0, len(all_kts), WIN):
                        wkts = all_kts[wstart : wstart + WIN]
                        ptiles = {}
                        for kt in wkts:
                            for s in slots:
                                if kt not in kt_sets[s]:
                                    continue
                                par = s % 2
                                rows = slice(64 * par, 64 * par + 64)
                                st = ps.tile([128, QC], f32, tag="s", bufs=4, name="st")
                                for hh in range(2):
                                    nc.tensor.matmul(
                                        st[:, hh * 256 : hh * 256 + 256],
                                        k2[pair][rows, kt * 128 : kt * 128 + 128],
                                        q2[pair][
                                            rows,
                                            qc * QC + hh * 256 : qc * QC + hh * 256 + 256,
                                        ],
                                        start=True,
                                        stop=True,
                                    )
                                off = 128 * kt - QC * qc
                                p_t = wp.tile([128, QC], f32r, tag="p", bufs=14, name="p_t")
                                if s in TPATH:
                                    j0 = -off - geom[s][0]
                                    nc.vector.tensor_tensor(
                                        st[:], st[:], tmpl_sb[s][:, j0 : j0 + QC], add_op
                                    )
                                    nc.scalar.activation(p_t[:], st[:], Exp)
                                else:
                                    z = max(0, off)
                                    if -127 <= off <= QC - 1:
                                        nc.vector.tensor_tensor(
                                            st[:, z : z + 128],
                                            st[:, z : z + 128],
                                            mdiag_sb[:],
                                            add_op,
                                        )
                                    fi = FPATH.index(s)
                                    nc.scalar.activation(
                                        p_t[:, z:QC],
                                        st[:, z:QC],
                                        Exp,
                                        bias=abias_sb[:, fi, qc, kt : kt + 1],
                                    )
                                    if z > 0:
                                        nc.vector.tensor_copy(
                                            p_t[:, 0:z], pzero_sb[:, 0:z]
                                        )
                                ptiles[(kt, s)] = p_t
                        for kt in wkts:
                            for s in slots:
                                if kt not in kt_sets[s]:
                                    continue
                                for hh in range(2):
                                    nc.tensor.matmul(
                                        pv_acc[(s, hh)][0:65, :],
                                        v_sb[:, kt, s, :],
                                        ptiles[(kt, s)][:, hh * 256 : hh * 256 + 256],
                                        start=(kt == kt_sets[s][0]),
                                        stop=(kt == kt_sets[s][-1]),
                                    )
                    # normalize & write O^T
                    for s in slots:
                        par = s % 2
                        den = np_.tile([128, QC], f32r, tag="den", name="den")
                        for hh in range(2):
                            nc.scalar.copy(
                                den[64:65, hh * 256 : hh * 256 + 256],
                                pv_acc[(s, hh)][64:65, :],
                            )
                        db = ps.tile([64, QC], f32, tag="s", bufs=4, name="db")
                        for hh in range(2):
                            nc.tensor.matmul(
                                db[:, hh * 256 : hh * 256 + 256],
                                vones_sb[64:65, 0:64],
                                den[64:65, hh * 256 : hh * 256 + 256],
                                start=True,
                                stop=True,
                            )
                        rb = np_.tile([64, QC], f32, tag="rb", name="rb")
                        nc.vector.reciprocal_approx_fast(rb[:], db[:])
                        if par == 0:
                            for hh in range(2):
                                nc.vector.tensor_tensor(
                                    ot[pair][
                                        0:64,
                                        qc * QC + hh * 256 : qc * QC + hh * 256 + 256,
                                    ],
                                    pv_acc[(s, hh)][0:64, :],
                                    rb[:, hh * 256 : hh * 256 + 256],
                                    mult_op,
                                )
                        else:
                            otmp = np_.tile([64, QC], f32r, tag="otmp", name="otmp")
                            for hh in range(2):
                                nc.vector.tensor_tensor(
                                    otmp[:, hh * 256 : hh * 256 + 256],
                                    pv_acc[(s, hh)][0:64, :],
                                    rb[:, hh * 256 : hh * 256 + 256],
                                    mult_op,
                                )
                            nc.sync.dma_start(ot[pair][64:128, qsl], otmp[:])

            # ---- out-projection (tail) ------------------------------
            if True:
                for lt in range(NLT):
                    for d2 in range(2):
                        dsl = slice(d2 * 512, d2 * 512 + 512)
                        po = ps.tile([128, 512], f32, tag="s", bufs=4, name="po")
                        for hh in range(2):
                            hso = slice(hh * 256, hh * 256 + 256)
                            hsw = slice(d2 * 512 + hh * 256, d2 * 512 + hh * 256 + 256)
                            nc.tensor.matmul(
                                po[:, hso], ot[0][:, lt * 128 : lt * 128 + 128],
                                wo_sb[:, 0, hsw], start=True, stop=False,
                            )
                            nc.tensor.matmul(
                                po[:, hso], ot[1][:, lt * 128 : lt * 128 + 128],
                                wo_sb[:, 1, hsw], start=False, stop=True,
                            )
                        o_sb = wp.tile([128, 512], f32, tag="osb", name="o_sb")
                        nc.vector.tensor_copy(o_sb[:], po[:])
                        nc.sync.dma_start(
                            out[lt * 128 : lt * 128 + 128, dsl], o_sb[:]
                        )
            np_.release()
            wp.release()

    nc.compile()
    return nc


def _get_nc(with_qb, with_kb, with_vb):
    global _NC
    key = (with_qb, with_kb, with_vb)
    if _NC is None or _NC[0] != key:
        _NC = (key, _build_nc(*key))
    return _NC[1]


def _round_fp32r(a):
    """Round fp32 -> fp32r (11 explicit mantissa bits, nearest-even)."""
    u = np.ascontiguousarray(a, np.float32).view(np.uint32)
    lsb = (u >> 12) & 1
    out = (((u + 0x7FF + lsb) >> 12) << 12).astype(np.uint32)
    return out.view(np.float32)


def make_in_maps(x, w_in, b_in, w_out):
    mdiag = np.where(
        np.arange(128)[:, None] <= np.arange(128)[None, :], 0.0, NEG
    ).astype(np.float32)
    in_maps = []
    for c in range(NCORES):
        b, g = c // 4, c % 4
        heads = [g, g + 4, g + 8, g + 12]
        cols = np.concatenate([np.arange(h * DH, h * DH + DH) for h in heads])
        xT = _round_fp32r(np.ascontiguousarray(x[b].T))
        wqm = _round_fp32r(w_in[:, cols] / 8.0)
        wkm = _round_fp32r(w_in[:, D + cols])
        wvm = _round_fp32r(w_in[:, 2 * D + cols])
        wom = _round_fp32r(w_out[cols, :])

        # template-path slots: bias+mask templates
        tm = {}
        for s in TPATH:
            c_min, w = _tmpl_geom(s)
            sl = SLOPES[4 * s + g]
            pk = np.arange(128)[:, None]
            cc = (np.arange(w) + c_min)[None, :]
            u = (pk - cc).astype(np.float32)
            tm[s] = np.where(u <= 0, sl * u, NEG).astype(np.float32)

        # free-path slots: per-partition ACT biases
        ab = np.zeros((128, len(FPATH), NQC, NKT), np.float32)
        for fi, s in enumerate(FPATH):
            sl = SLOPES[4 * s + g]
            for qc in range(NQC):
                for kt in _kt_range(s, qc):
                    ab[:, fi, qc, kt] = sl * (
                        128 * kt + np.arange(128) - (QC * qc + QC - 1)
                    )
        m = {
            "xT": xT, "wq": wqm, "wk": wkm, "wv": wvm, "wo": wom,
            "abias": ab, "mdiag": mdiag,
            "vones": np.ones((128, NKT, 4), np.float32),
            "pzero": np.zeros((128, 384), np.float32),
        }
        for s in TPATH:
            m[f"tmpl{s}"] = tm[s]
        if np.any(b_in[cols]):
            qbias = (b_in[cols] / 8.0).astype(np.float32)
            m["qb"] = np.stack([qbias[:128], qbias[128:]], axis=1)
        if np.any(b_in[D + cols]):
            kbias = b_in[D + cols].astype(np.float32)
            m["kb"] = np.stack([kbias[:128], kbias[128:]], axis=1)
        if np.any(b_in[2 * D + cols]):
            m["vb"] = np.broadcast_to(
                b_in[2 * D + cols].astype(np.float32), (128, 256)
            ).copy()
        in_maps.append(m)
    return in_maps


def kernel(x, w_in, b_in, w_out, b_out):
    from concourse.bass_utils import run_bass_kernel_spmd

    x = np.asarray(x, np.float32)
    w_in = np.asarray(w_in, np.float32)
    b_in = np.asarray(b_in, np.float32)
    w_out = np.asarray(w_out, np.float32)
    b_out = np.asarray(b_out, np.float32)
    assert x.shape == (B, L, D) and w_in.shape == (D, 3 * D)

    in_maps = make_in_maps(x, w_in, b_in, w_out)
    nc = _get_nc("qb" in in_maps[0], "kb" in in_maps[0], "vb" in in_maps[0])
    res = run_bass_kernel_spmd(nc, in_maps, core_ids=list(range(NCORES)))
    out = np.zeros((B, L, D), np.float32)
    for c in range(NCORES):
        out[c // 4] += res.results[c]["out"]
    out += b_out[None, None, :]
    return out


# revision 23
# speedup vs baseline: 1.0401x; 1.0401x over previous
"""ALiBi causal self-attention on 8 TRN2 NeuronCores.

Sharding: core c handles batch b = c // 4 and the 4 heads {g, g+4, g+8, g+12}
with g = c % 4 (one head per "slope band" so per-core work balances and the
SPMD program is slot-uniform across cores).

Per-core program (single SPMD NEFF, fp32 storage, float32r matmuls):
  1. qkv^T = W^T x^T (2-head-paired matmuls, contraction over D on partitions)
  2. attention in transposed layout S^T[k, q] per 512-wide q-chunk with banded
     causal k-tiles; ALiBi bias via
       - slots 0/1 (largest slopes): exact bias+mask template added on DVE
       - slots 2/3: per-partition ACT bias exp(s + slope*(k - qmax)); the
         column factor exp(-slope*(q - qmax)) cancels in the softmax
         normalization, so it is never computed
     softmax denominator comes free from a ones-column appended to V
  3. normalize: broadcast the denominator row across partitions with a K=1
     matmul, reciprocal_approx_fast on the broadcast, one DVE multiply;
     then the out-projection (row-parallel); host sums the per-batch partials.
"""

import math

import numpy as np

B, L, D, H = 2, 2048, 1024, 16
DH = D // H  # 64
NCORES = 8
QC = 512  # q-chunk width
NQC = L // QC  # 4
NKT = L // 128  # 16 k-tiles
NLT = L // 128  # 16 l-tiles
NDC = D // 128  # 8 contraction chunks
NEG = -1.0e9
BAND_T = 20.0  # drop tiles whose best entry has bias < -BAND_T (exp < 2e-9)
TPATH = (0, 1)  # template-path slots
FPATH = (2, 3)  # free-path slots (ACT per-partition bias)

_NC = None


def _alibi_slopes(n):
    def pow2_slopes(m):
        start = 2 ** (-(2 ** -(math.log2(m) - 3)))
        return [start * start**i for i in range(m)]

    if math.log2(n).is_integer():
        return pow2_slopes(n)
    c = 2 ** math.floor(math.log2(n))
    return pow2_slopes(c) + _alibi_slopes(2 * c)[0::2][: n - c]


SLOPES = _alibi_slopes(H)
# Band window per slot = max over the heads that can occupy it (head 4j+g).
_W = [int(math.ceil(max(BAND_T / SLOPES[4 * j + g] for g in range(4)))) for j in range(4)]
WSLOT = [w if w <= L else 10**9 for w in _W]


def _kt_range(slot, qc):
    lo = max(0, -(-(QC * qc - 127 - WSLOT[slot]) // 128))
    return range(lo, 4 * qc + 4)


def _tmpl_geom(slot):
    """Template column geometry for a template-path slot: col j <-> c =
    j + c_min; tile (kt, qc) entry [pk, fq] reads c = fq - off with
    off = 128*kt - QC*qc."""
    offs = [128 * kt - QC * qc for qc in range(NQC) for kt in _kt_range(slot, qc)]
    c_min = 0 - max(offs)
    c_max = QC - 1 - min(offs)
    return c_min, c_max - c_min + 1


def _build_nc(with_qb, with_kb, with_vb):
    import concourse.bacc as bacc
    import concourse.mybir as mybir
    import concourse.tile as tile

    geom = {s: _tmpl_geom(s) for s in TPATH}

    f32 = mybir.dt.float32
    f32r = mybir.dt.float32r

    nc = bacc.Bacc("TRN2", target_bir_lowering=False, debug=False)

    xT = nc.dram_tensor("xT", [D, L], f32r, kind="ExternalInput")
    wq = nc.dram_tensor("wq", [D, 256], f32r, kind="ExternalInput")
    wk = nc.dram_tensor("wk", [D, 256], f32r, kind="ExternalInput")
    wv = nc.dram_tensor("wv", [D, 256], f32r, kind="ExternalInput")
    wo = nc.dram_tensor("wo", [256, D], f32r, kind="ExternalInput")
    tmpl = {
        s: nc.dram_tensor(f"tmpl{s}", [128, geom[s][1]], f32, kind="ExternalInput")
        for s in TPATH
    }
    abias = nc.dram_tensor(
        "abias", [128, len(FPATH), NQC, NKT], f32, kind="ExternalInput"
    )
    mdiag = nc.dram_tensor("mdiag", [128, 128], f32, kind="ExternalInput")
    vones = nc.dram_tensor("vones", [128, NKT, 4], f32r, kind="ExternalInput")
    pzero = nc.dram_tensor("pzero", [128, 384], f32r, kind="ExternalInput")
    if with_qb:
        qb = nc.dram_tensor("qb", [128, 2], f32, kind="ExternalInput")
    if with_kb:
        kb = nc.dram_tensor("kb", [128, 2], f32, kind="ExternalInput")
    if with_vb:
        vb = nc.dram_tensor("vb", [128, 256], f32, kind="ExternalInput")
    out = nc.dram_tensor("out", [L, D], f32, kind="ExternalOutput")

    Exp = mybir.ActivationFunctionType.Exp
    Identity = mybir.ActivationFunctionType.Identity
    add_op = mybir.AluOpType.add
    mult_op = mybir.AluOpType.mult

    with tile.TileContext(nc) as tc:
        with (
            tc.tile_pool(name="persist", bufs=1) as pp,
            tc.tile_pool(name="psum", bufs=1, space="PSUM") as ps,
        ):
            # ---- load constants / weights -------------------------------
            wq_sb = pp.tile([128, NDC, 256], f32r, tag="wq")
            wk_sb = pp.tile([128, NDC, 256], f32r, tag="wk")
            wv_sb = pp.tile([128, NDC, 256], f32r, tag="wv")
            wo_sb = pp.tile([128, 2, D], f32r, tag="wo")
            nc.sync.dma_start(wq_sb[:], wq.ap().rearrange("(c p) m -> p c m", p=128))
            nc.sync.dma_start(wk_sb[:], wk.ap().rearrange("(c p) m -> p c m", p=128))
            nc.sync.dma_start(wv_sb[:], wv.ap().rearrange("(c p) m -> p c m", p=128))
            nc.sync.dma_start(wo_sb[:], wo.ap().rearrange("(c p) m -> p c m", p=128))
            tmpl_sb = {}
            for s in TPATH:
                t = pp.tile([128, geom[s][1]], f32, tag=f"tmpl{s}", name=f"tmpl{s}")
                nc.sync.dma_start(t[:], tmpl[s][:])
                tmpl_sb[s] = t
            abias_sb = pp.tile([128, len(FPATH), NQC, NKT], f32, tag="abias")
            nc.sync.dma_start(abias_sb[:], abias[:])
            mdiag_sb = pp.tile([128, 128], f32, tag="mdiag")
            nc.sync.dma_start(mdiag_sb[:], mdiag[:])
            pzero_sb = pp.tile([128, 384], f32r, tag="pzero")
            nc.sync.dma_start(pzero_sb[:], pzero[:])
            vones_sb = pp.tile([128, NKT * 4], f32r, tag="vones")
            nc.sync.dma_start(vones_sb[:], vones.ap().rearrange("p a b -> p (a b)"))
            if with_qb:
                qb_sb = pp.tile([128, 2], f32, tag="qb")
                nc.sync.dma_start(qb_sb[:], qb[:])
            if with_kb:
                kb_sb = pp.tile([128, 2], f32, tag="kb")
                nc.sync.dma_start(kb_sb[:], kb[:])
            if with_vb:
                vb_sb = pp.tile([128, 256], f32, tag="vb")
                nc.sync.dma_start(vb_sb[:], vb[:])

            xp = tc.alloc_tile_pool(name="xt", bufs=1)
            xt_sb = []
            for dc in range(NDC):
                t = xp.tile([128, L], f32r, tag=f"xt{dc}", name=f"xt{dc}")
                nc.sync.dma_start(t[:], xT[dc * 128 : (dc + 1) * 128, :])
                xt_sb.append(t)

            # ---- persistent intermediates -------------------------------
            q2 = [pp.tile([128, L], f32r, tag=f"q2_{p}", name=f"q2_{p}") for p in range(2)]
            k2 = [pp.tile([128, L], f32r, tag=f"k2_{p}", name=f"k2_{p}") for p in range(2)]
            v_sb = pp.tile([128, NKT, 4, 65], f32r, tag="v")
            ot = [pp.tile([128, L], f32r, tag=f"ot_{p}", name=f"ot_{p}") for p in range(2)]

            # ---- QKV projections ---------------------------------------
            for pair in range(2):
                for qc in range(NQC):
                    sl = slice(qc * 512, qc * 512 + 512)
                    pq = ps.tile([128, 512], f32, tag="s", bufs=4, name="pq")
                    for hh in range(2):
                        hsl = slice(qc * 512 + hh * 256, qc * 512 + hh * 256 + 256)
                        for dc in range(NDC):
                            nc.tensor.matmul(
                                pq[:, hh * 256 : hh * 256 + 256],
                                wq_sb[:, dc, pair * 128 : pair * 128 + 128],
                                xt_sb[dc][:, hsl],
                                start=(dc == 0),
                                stop=(dc == NDC - 1),
                            )
                    if with_qb:
                        nc.scalar.activation(
                            q2[pair][:, sl], pq[:], Identity,
                            bias=qb_sb[:, pair : pair + 1],
                        )
                    else:
                        nc.scalar.copy(q2[pair][:, sl], pq[:])
                    pk_ = ps.tile([128, 512], f32, tag="s", bufs=4, name="pk_")
                    for hh in range(2):
                        hsl = slice(qc * 512 + hh * 256, qc * 512 + hh * 256 + 256)
                        for dc in range(NDC):
                            nc.tensor.matmul(
                                pk_[:, hh * 256 : hh * 256 + 256],
                                wk_sb[:, dc, pair * 128 : pair * 128 + 128],
                                xt_sb[dc][:, hsl],
                                start=(dc == 0),
                                stop=(dc == NDC - 1),
                            )
                    if with_kb:
                        nc.scalar.activation(
                            k2[pair][:, sl], pk_[:], Identity,
                            bias=kb_sb[:, pair : pair + 1],
                        )
                    else:
                        nc.scalar.copy(k2[pair][:, sl], pk_[:])

            for lt in range(NLT):
                pv_ = ps.tile([128, 256], f32, tag="s", bufs=4, name="pv_")
                for dc in range(NDC):
                    nc.tensor.matmul(
                        pv_[:],
                        xt_sb[dc][:, lt * 128 : lt * 128 + 128],
                        wv_sb[:, dc, :],
                        start=(dc == 0),
                        stop=(dc == NDC - 1),
                    )
                if with_vb:
                    nc.vector.tensor_tensor(pv_[:], pv_[:], vb_sb[:], add_op)
                nc.vector.tensor_copy(
                    v_sb[:, lt, 0:4, 0:64],
                    pv_.rearrange("p (s m) -> p s m", s=4),
                )
            nc.sync.dma_start(
                v_sb[:, :, :, 64:65], vones.ap().rearrange("p a (b o) -> p a b o", o=1)
            )
            xp.release()
            wp = tc.alloc_tile_pool(name="work", bufs=8)
            np_ = tc.alloc_tile_pool(name="norm", bufs=3)

            # ---- attention ----------------------------------------------
            for qc in range(NQC):
                qsl = slice(qc * QC, qc * QC + QC)
                for pair in range(2):
                    slots = (2 * pair, 2 * pair + 1)
                    kt_sets = {s: list(_kt_range(s, qc)) for s in slots}
                    pv_acc = {
                        (s, hh): ps.tile(
                            [128, 256], f32, tag="pv", bufs=4, name=f"pvacc{s}_{hh}"
                        )
                        for s in slots
                        for hh in range(2)
                    }
                    all_kts = sorted(set(kt_sets[slots[0]]) | set(kt_sets[slots[1]]))
                    WIN = 6
                    for wstart in range(0, len(all_kts), WIN):
                        wkts = all_kts[wstart : wstart + WIN]
                        ptiles = {}
                        for kt in wkts:
                            for s in slots:
                                if kt not in kt_sets[s]:
                                    continue
                                par = s % 2
                                rows = slice(64 * par, 64 * par + 64)
                                st = ps.tile([128, QC], f32, tag="s", bufs=4, name="st")
                                for hh in range(2):
                                    nc.tensor.matmul(
                                        st[:, hh * 256 : hh * 256 + 256],
                                        k2[pair][rows, kt * 128 : kt * 128 + 128],
                                        q2[pair][
                                            rows,
                                            qc * QC + hh * 256 : qc * QC + hh * 256 + 256,
                                        ],
                                        start=True,
                                        stop=True,
                                    )
                                off = 128 * kt - QC * qc
                                p_t = wp.tile([128, QC], f32r, tag="p", bufs=14, name="p_t")
                                if s in TPATH:
                                    j0 = -off - geom[s][0]
                                    nc.vector.tensor_tensor(
                                        st[:], st[:], tmpl_sb[s][:, j0 : j0 + QC], add_op
                                    )
                                    nc.scalar.activation(p_t[:], st[:], Exp)
                                else:
                                    z = max(0, off)
                                    if -127 <= off <= QC - 1:
                                        nc.vector.tensor_tensor(
                                            st[:, z : z + 128],
                                            st[:, z : z + 128],
                                            mdiag_sb[:],
                                            add_op,
                                        )
                                    fi = FPATH.index(s)
                                    nc.scalar.activation(
                                        p_t[:, z:QC],
                                        st[:, z:QC],
                                        Exp,
                                        bias=abias_sb[:, fi, qc, kt : kt + 1],
                                    )
                                    if z > 0:
                                        nc.vector.tensor_copy(
                                            p_t[:, 0:z], pzero_sb[:, 0:z]
                                        )
                                ptiles[(kt, s)] = p_t
                        for kt in wkts:
                            for s in slots:
                                if kt not in kt_sets[s]:
                                    continue
                                for hh in range(2):
                                    nc.tensor.matmul(
                                        pv_acc[(s, hh)][0:65, :],
                                        v_sb[:, kt, s, :],
                                        ptiles[(kt, s)][:, hh * 256 : hh * 256 + 256],
                                        start=(kt == kt_sets[s][0]),
                                        stop=(kt == kt_sets[s][-1]),
                                    )
                    # normalize & write O^T
                    for s in slots:
                        par = s % 2
                        den = np_.tile([128, QC], f32r, tag="den", name="den")
                        for hh in range(2):
                            nc.scalar.copy(
                                den[64:65, hh * 256 : hh * 256 + 256],
                                pv_acc[(s, hh)][64:65, :],
                            )
                        db = ps.tile([64, QC], f32, tag="s", bufs=4, name="db")
                        for hh in range(2):
                            nc.tensor.matmul(
                                db[:, hh * 256 : hh * 256 + 256],
                                vones_sb[64:65, 0:64],
                                den[64:65, hh * 256 : hh * 256 + 256],
                                start=True,
                                stop=True,
                            )
                        rb = np_.tile([64, QC], f32, tag="rb", name="rb")
                        nc.vector.reciprocal_approx_fast(rb[:], db[:])
                        if par == 0:
                            for hh in range(2):
                                nc.vector.tensor_tensor(
                                    ot[pair][
                                        0:64,
                                        qc * QC + hh * 256 : qc * QC + hh * 256 + 256,
                                    ],
                                    pv_acc[(s, hh)][0:64, :],
                                    rb[:, hh * 256 : hh * 256 + 256],
                                    mult_op,
                                )
                        else:
                            otmp = np_.tile([64, QC], f32r, tag="otmp", name="otmp")
                            for hh in range(2):
                                nc.vector.tensor_tensor(
                                    otmp[:, hh * 256 : hh * 256 + 256],
                                    pv_acc[(s, hh)][0:64, :],
                                    rb[:, hh * 256 : hh * 256 + 256],
                                    mult_op,
                                )
                            nc.sync.dma_start(ot[pair][64:128, qsl], otmp[:])

            # ---- out-projection (tail) ------------------------------
            if True:
                for lt in range(NLT):
                    for d2 in range(2):
                        dsl = slice(d2 * 512, d2 * 512 + 512)
                        po = ps.tile([128, 512], f32, tag="s", bufs=4, name="po")
                        for hh in range(2):
                            hso = slice(hh * 256, hh * 256 + 256)
                            hsw = slice(d2 * 512 + hh * 256, d2 * 512 + hh * 256 + 256)
                            nc.tensor.matmul(
                                po[:, hso], ot[0][:, lt * 128 : lt * 128 + 128],
                                wo_sb[:, 0, hsw], start=True, stop=False,
                            )
                            nc.tensor.matmul(
                                po[:, hso], ot[1][:, lt * 128 : lt * 128 + 128],
                                wo_sb[:, 1, hsw], start=False, stop=True,
                            )
                        o_sb = wp.tile([128, 512], f32, tag="osb", name="o_sb")
                        nc.vector.tensor_copy(o_sb[:], po[:])
                        nc.sync.dma_start(
                            out[lt * 128 : lt * 128 + 128, dsl], o_sb[:]
                        )
            np_.release()
            wp.release()

    nc.compile()
    return nc


def _get_nc(with_qb, with_kb, with_vb):
    global _NC
    key = (with_qb, with_kb, with_vb)
    if _NC is None or _NC[0] != key:
        _NC = (key, _build_nc(*key))
    return _NC[1]


def _round_fp32r(a):
    """Round fp32 -> fp32r (11 explicit mantissa bits, nearest-even)."""
    u = np.ascontiguousarray(a, np.float32).view(np.uint32)
    lsb = (u >> 12) & 1
    out = (((u + 0x7FF + lsb) >> 12) << 12).astype(np.uint32)
    return out.view(np.float32)


def make_in_maps(x, w_in, b_in, w_out):
    mdiag = np.where(
        np.arange(128)[:, None] <= np.arange(128)[None, :], 0.0, NEG
    ).astype(np.float32)
    in_maps = []
    for c in range(NCORES):
        b, g = c // 4, c % 4
        heads = [g, g + 4, g + 8, g + 12]
        cols = np.concatenate([np.arange(h * DH, h * DH + DH) for h in heads])
        xT = _round_fp32r(np.ascontiguousarray(x[b].T))
        wqm = _round_fp32r(w_in[:, cols] / 8.0)
        wkm = _round_fp32r(w_in[:, D + cols])
        wvm = _round_fp32r(w_in[:, 2 * D + cols])
        wom = _round_fp32r(w_out[cols, :])

        # template-path slots: bias+mask templates
        tm = {}
        for s in TPATH:
            c_min, w = _tmpl_geom(s)
            sl = SLOPES[4 * s + g]
            pk = np.arange(128)[:, None]
            cc = (np.arange(w) + c_min)[None, :]
            u = (pk - cc).astype(np.float32)
            tm[s] = np.where(u <= 0, sl * u, NEG).astype(np.float32)

        # free-path slots: per-partition ACT biases
        ab = np.zeros((128, len(FPATH), NQC, NKT), np.float32)
        for fi, s in enumerate(FPATH):
            sl = SLOPES[4 * s + g]
            for qc in range(NQC):
                for kt in _kt_range(s, qc):
                    ab[:, fi, qc, kt] = sl * (
                        128 * kt + np.arange(128) - (QC * qc + QC - 1)
                    )
        m = {
            "xT": xT, "wq": wqm, "wk": wkm, "wv": wvm, "wo": wom,
            "abias": ab, "mdiag": mdiag,
            "vones": np.ones((128, NKT, 4), np.float32),
            "pzero": np.zeros((128, 384), np.float32),
        }
        for s in TPATH:
            m[f"tmpl{s}"] = tm[s]
        if np.any(b_in[cols]):
            qbias = (b_in[cols] / 8.0).astype(np.float32)
            m["qb"] = np.stack([qbias[:128], qbias[128:]], axis=1)
        if np.any(b_in[D + cols]):
            kbias = b_in[D + cols].astype(np.float32)
            m["kb"] = np.stack([kbias[:128], kbias[128:]], axis=1)
        if np.any(b_in[2 * D + cols]):
            m["vb"] = np.broadcast_to(
                b_in[2 * D + cols].astype(np.float32), (128, 256)
            ).copy()
        in_maps.append(m)
    return in_maps


def kernel(x, w_in, b_in, w_out, b_out):
    from concourse.bass_utils import run_bass_kernel_spmd

    x = np.asarray(x, np.float32)
    w_in = np.asarray(w_in, np.float32)
    b_in = np.asarray(b_in, np.float32)
    w_out = np.asarray(w_out, np.float32)
    b_out = np.asarray(b_out, np.float32)
    assert x.shape == (B, L, D) and w_in.shape == (D, 3 * D)

    in_maps = make_in_maps(x, w_in, b_in, w_out)
    nc = _get_nc("qb" in in_maps[0], "kb" in in_maps[0], "vb" in in_maps[0])
    res = run_bass_kernel_spmd(nc, in_maps, core_ids=list(range(NCORES)))
    out = np.zeros((B, L, D), np.float32)
    for c in range(NCORES):
        out[c // 4] += res.results[c]["out"]
    out += b_out[None, None, :]
    return out
